# revision 6
# baseline (speedup 1.0000x reference)
"""GAT message-passing kernel for 8 Trainium2 NeuronCores.

Problem (hardcoded shapes): B=4, N=4096, Cin=200, HC=128.
    x   = rm @ W.T + b                      (B, N, HC)
    e   = (x@a_src)[:, :, None] + (x@a_dst)[:, None, :]
    e   = leaky_relu(e * adj, 0.2)
    out = softmax(e, -1) @ x                (B, N, HC)

Sharding: 8 cores = batch (4) x query-row halves (2). Each core owns
adj[b, i0:i0+2048, :] (33.5 MB) and produces out[b, i0:i0+2048, :].

Per-core algorithm (transposed score layout [j partitions, i free]):
  - scores decompose: src_i = rm_i . (W.T a_src) + b.(a_src+a_dst),
    dst_j = rm_j . (W.T a_dst);  e_ij = src_i + dst_j.
  - leaky_relu is positively homogeneous and adj in {0,1}, so
    lrelu(e*adj) = adj * lrelu(e): one fused custom DVE op computes
    m = adjT * lrelu(src_bcast + dst) in a single 1x pass.
  - w = exp(m) on ScalarE (exp(0)=1 for masked entries matches the
    reference's mask-by-multiplication semantics).
  - aggregation: U[i, 0:128] = sum_j w_ij * x[j, :], D[i] = U[i, 128]
    via one bf16 matmul chain against x~ = [x | 1]; out = U / D.
  - adj is cast fp32->bf16 during the DMA load (values 0/1 exact) and
    transposed on the PE (1 cyc/row) into PSUM for the DVE mask op.
"""

import sys

if "/opt/trn_rl_repo" not in sys.path:
    sys.path.insert(0, "/opt/trn_rl_repo")

import numpy as np

B, N, Cin, HC = 4, 4096, 200, 128
ROWS = N // 2  # rows of adj per core
NEG_SLOPE = 0.2
NCORES = 8
C1 = 128              # first Cin chunk
C2 = Cin - C1         # second Cin chunk (72)
XW = HC + 1           # x~ width (129)
NJT = N // 128        # 32 j-tiles
PANEL = 512
NPANEL = ROWS // PANEL  # 4
NOWN = ROWS // 128      # 16 own i-tiles

_CACHE = {}


def _register_custom_op():
    """Fused DVE op: out = in1 * leaky_relu(in0 + s0, slope=s1).

    in0 = src broadcast tile (stream), s0 = dst per-partition scalar,
    in1 = transposed adj tile (stream, PSUM), s1 = slope immediate.
    """
    import concourse.dve_ops as dve_ops
    from concourse.dve_spec import Spec, Src0, Src1, C0, C1 as SC1, maxx, lower
    from concourse.dve_uop import DveOpSpec

    name = "GAT_MASKED_LRELU_ANT"
    for op in dve_ops.OPS:
        if op.name == name:
            return op
    _t = Src0 + C0
    body = maxx(_t, _t * SC1) * Src1
    spec = Spec(
        body=body,
        reference=lambda in0, in1, s0, s1, imm2: np.maximum(
            in0 + s0, (in0 + s0) * s1
        )
        * in1,
    )
    row = dve_ops._CUSTOM_DVE_ROW_BASE + len(dve_ops.OPS)
    shas = {}
    for ver in ("v3", "v4"):
        uops = lower(spec, ver=ver)
        shas[ver] = DveOpSpec(name=name, opcode=row, uops=uops, rd1_en=True).sha(ver)
    op = dve_ops.DveOp(name, spec, subdim=False, uops_sha=shas)
    dve_ops.OPS.append(op)
    dve_ops._SUB_OPCODE_FOR_NAME[name] = row
    dve_ops.CUSTOM_DVE_SPECS[name] = spec
    return op


def _build():
    if "nc" in _CACHE:
        return _CACHE["nc"]

    import concourse.mybir as mybir
    import concourse.tile as tile
    from concourse import bacc
    from concourse.masks import make_identity

    GAT_OP = _register_custom_op()
    f32 = mybir.dt.float32
    bf16 = mybir.dt.bfloat16
    AF = mybir.ActivationFunctionType

    nc = bacc.Bacc("TRN2", target_bir_lowering=False, debug=False, num_devices=NCORES)

    rm = nc.dram_tensor("rm", [N, Cin], f32, kind="ExternalInput").ap()
    rmo = nc.dram_tensor("rmo", [ROWS, Cin], f32, kind="ExternalInput").ap()
    adjs = nc.dram_tensor("adjs", [ROWS, N], f32, kind="ExternalInput").ap()
    Wd = nc.dram_tensor("Wt", [HC, Cin], f32, kind="ExternalInput").ap()
    brow = nc.dram_tensor("brow", [1, XW + 1], f32, kind="ExternalInput").ap()
    bcol = nc.dram_tensor("bcol", [HC, 1], f32, kind="ExternalInput").ap()
    asrc = nc.dram_tensor("asrc", [HC, 1], f32, kind="ExternalInput").ap()
    adst = nc.dram_tensor("adst", [HC, 1], f32, kind="ExternalInput").ap()
    outd = nc.dram_tensor("out", [ROWS, HC], f32, kind="ExternalOutput").ap()

    with tile.TileContext(nc) as tc:
        with (
            tc.tile_pool(name="const", bufs=1) as cp,
            tc.tile_pool(name="persist", bufs=1) as pp,
        ):
            # ---------- constants ----------
            ident_f = cp.tile([128, 128], f32)
            make_identity(nc, ident_f[:])
            ident_b = cp.tile([128, 128], bf16)
            nc.vector.tensor_copy(ident_b[:], ident_f[:])
            ones_row = cp.tile([1, 128], f32)
            nc.gpsimd.memset(ones_row[:], 1.0)
            W_sb = cp.tile([128, Cin], f32)
            nc.sync.dma_start(out=W_sb[:], in_=Wd)
            brow_sb = cp.tile([1, XW + 1], f32)
            nc.sync.dma_start(out=brow_sb[:], in_=brow)
            bcol_sb = cp.tile([HC, 1], f32)
            nc.sync.dma_start(out=bcol_sb[:], in_=bcol)
            asrc_sb = cp.tile([HC, 1], f32)
            nc.sync.dma_start(out=asrc_sb[:], in_=asrc)
            adst_sb = cp.tile([HC, 1], f32)
            nc.sync.dma_start(out=adst_sb[:], in_=adst)

            with tc.tile_pool(name="setup_ps", bufs=2, space="PSUM") as sps:
                # W^T (two chunks packed side by side: [c0:128, d] | [c128:200, d])
                wt_ps = sps.tile([128, 256], f32, tag="wt")
                nc.tensor.transpose(wt_ps[:, 0:128], W_sb[:, 0:C1], ident_f[:])
                nc.tensor.transpose(wt_ps[0:C2, 128:256], W_sb[:, C1:Cin], ident_f[:])
                WT_sb = cp.tile([128, 256], f32)
                nc.scalar.copy(WT_sb[:, 0:128], wt_ps[:, 0:128])
                nc.scalar.copy(WT_sb[0:C2, 128:256], wt_ps[0:C2, 128:256])

                # w_src / w_dst = W^T a (cols: src_lo, src_hi, dst_lo, dst_hi)
                wv_ps = sps.tile([128, 4], f32, tag="wv")
                nc.tensor.matmul(wv_ps[:, 0:1], W_sb[:, 0:C1], asrc_sb[:], start=True, stop=True)
                nc.tensor.matmul(wv_ps[0:C2, 1:2], W_sb[:, C1:Cin], asrc_sb[:], start=True, stop=True)
                nc.tensor.matmul(wv_ps[:, 2:3], W_sb[:, 0:C1], adst_sb[:], start=True, stop=True)
                nc.tensor.matmul(wv_ps[0:C2, 3:4], W_sb[:, C1:Cin], adst_sb[:], start=True, stop=True)
                wv_sb = cp.tile([128, 4], f32)
                nc.scalar.copy(wv_sb[:, 0:1], wv_ps[:, 0:1])
                nc.scalar.copy(wv_sb[0:C2, 1:2], wv_ps[0:C2, 1:2])
                nc.scalar.copy(wv_sb[:, 2:3], wv_ps[:, 2:3])
                nc.scalar.copy(wv_sb[0:C2, 3:4], wv_ps[0:C2, 3:4])

                # scalar const C = b.(a_src + a_dst), folded into src
                c_ps = sps.tile([1, 1], f32, tag="c")
                nc.tensor.matmul(c_ps[:], bcol_sb[:], asrc_sb[:], start=True, stop=False)
                nc.tensor.matmul(c_ps[:], bcol_sb[:], adst_sb[:], start=False, stop=True)
                c_sb = cp.tile([1, 1], f32)
                nc.scalar.copy(c_sb[:], c_ps[:])

            # ---------- x~, dst, src ----------
            xt_all = pp.tile([128, NJT * XW], bf16)   # x~ = [x | 1] per j-tile
            dst_all = pp.tile([128, NJT], f32)        # dst column per j-tile
            src_row = pp.tile([1, ROWS], f32)
            src_bc = pp.tile([128, ROWS], f32)        # src broadcast along partitions

            with (
                tc.tile_pool(name="xin", bufs=3) as xp,
                tc.tile_pool(name="xT", bufs=2) as xtp,
                tc.tile_pool(name="x_ps", bufs=2, space="PSUM") as xps,
                tc.tile_pool(name="x_ps2", bufs=2, space="PSUM") as xps2,
            ):
                for n in range(NJT):
                    rm_t = xp.tile([128, Cin], f32, tag="rm")
                    nc.sync.dma_start(out=rm_t[:], in_=rm[n * 128:(n + 1) * 128, :])
                    rT_ps = xps.tile([128, 256], f32, tag="rT")
                    nc.tensor.transpose(rT_ps[:, 0:128], rm_t[:, 0:C1], ident_f[:])
                    nc.tensor.transpose(rT_ps[0:C2, 128:256], rm_t[:, C1:Cin], ident_f[:])
                    rT_sb = xtp.tile([128, 256], f32, tag="rTs")
                    nc.scalar.copy(rT_sb[:, 0:128], rT_ps[:, 0:128])
                    nc.scalar.copy(rT_sb[0:C2, 128:256], rT_ps[0:C2, 128:256])

                    # cols 0:129 = x~ = [x | 1]; col 129 = dst.  One
                    # accumulation group: the K=1 bias matmul seeds every
                    # column (b | 1 | 0), everything else accumulates.
                    x_ps = xps2.tile([128, XW + 1], f32, tag="xps")
                    nc.tensor.matmul(x_ps[:], ones_row[:], brow_sb[:], start=True, stop=False)
                    nc.tensor.matmul(x_ps[:, 0:HC], rT_sb[:, 0:128], WT_sb[:, 0:128], start=False, stop=False)
                    nc.tensor.matmul(x_ps[:, 0:HC], rT_sb[0:C2, 128:256], WT_sb[0:C2, 128:256], start=False, stop=False)
                    nc.tensor.matmul(x_ps[:, XW:XW + 1], rT_sb[:, 0:128], wv_sb[:, 2:3], start=False, stop=False)
                    nc.tensor.matmul(x_ps[:, XW:XW + 1], rT_sb[0:C2, 128:256], wv_sb[0:C2, 3:4], start=False, stop=True)
                    nc.vector.tensor_copy(xt_all[:, n * XW:(n + 1) * XW], x_ps[:, 0:XW])
                    nc.vector.tensor_copy(dst_all[:, n:n + 1], x_ps[:, XW:XW + 1])

                for k in range(NOWN):
                    ro_t = xp.tile([128, Cin], f32, tag="rm")
                    nc.sync.dma_start(out=ro_t[:], in_=rmo[k * 128:(k + 1) * 128, :])
                    roT_ps = xps.tile([128, 256], f32, tag="rT")
                    nc.tensor.transpose(roT_ps[:, 0:128], ro_t[:, 0:C1], ident_f[:])
                    nc.tensor.transpose(roT_ps[0:C2, 128:256], ro_t[:, C1:Cin], ident_f[:])
                    roT_sb = xtp.tile([128, 256], f32, tag="rTs")
                    nc.scalar.copy(roT_sb[:, 0:128], roT_ps[:, 0:128])
                    nc.scalar.copy(roT_sb[0:C2, 128:256], roT_ps[0:C2, 128:256])

                    s_ps = xps2.tile([1, 128], f32, tag="xps", name="s_ps")
                    nc.tensor.matmul(s_ps[:], wv_sb[:, 0:1], roT_sb[:, 0:128], start=True, stop=False)
                    nc.tensor.matmul(s_ps[:], wv_sb[0:C2, 1:2], roT_sb[0:C2, 128:256], start=False, stop=False)
                    nc.tensor.matmul(s_ps[:], c_sb[:], ones_row[:], start=False, stop=True)
                    nc.scalar.copy(src_row[:, k * 128:(k + 1) * 128], s_ps[:])

                for q in range(ROWS // 512):
                    sb_ps = xps.tile([128, 512], f32, tag="rT", name="sb_ps")
                    nc.tensor.matmul(sb_ps[:], ones_row[:], src_row[:, q * 512:(q + 1) * 512], start=True, stop=True)
                    nc.vector.tensor_copy(src_bc[:, q * 512:(q + 1) * 512], sb_ps[:])

            # ---------- main loop ----------
            with (
                tc.tile_pool(name="adj", bufs=8) as adjp,
                tc.tile_pool(name="mbuf", bufs=2) as mwp,
                tc.tile_pool(name="wbuf", bufs=2) as wxp,
                tc.tile_pool(name="fin", bufs=4) as finp,
                tc.tile_pool(name="U_ps", bufs=4, space="PSUM") as upsp,
                tc.tile_pool(name="aT_ps", bufs=2, space="PSUM") as atp,
            ):
                for p in range(NPANEL):
                    strips = []
                    for s in range(4):
                        at = adjp.tile([128, N], bf16, tag="adj")
                        r0 = p * PANEL + s * 128
                        nc.gpsimd.dma_start(out=at[:], in_=adjs[r0:r0 + 128, :])
                        strips.append(at)
                    Us = [
                        upsp.tile([128, XW], f32, tag="U", name=f"U_{p}_{i}")
                        for i in range(4)
                    ]
                    for jt2 in range(NJT // 2):
                        m_t = mwp.tile([128, 1024], f32, tag="m")
                        for h in range(2):
                            jt = jt2 * 2 + h
                            aT = atp.tile([128, PANEL], bf16, tag="aT")
                            for s in range(4):
                                nc.tensor.transpose(
                                    aT[:, s * 128:(s + 1) * 128],
                                    strips[s][:, jt * 128:(jt + 1) * 128],
                                    ident_b[:],
                                )
                            nc.vector._custom_dve(
                                GAT_OP,
                                out=m_t[:, h * PANEL:(h + 1) * PANEL],
                                in0=src_bc[:, p * PANEL:(p + 1) * PANEL],
                                in1=aT[:],
                                s0=dst_all[:, jt:jt + 1],
                                s1=NEG_SLOPE,
                            )
                        w_t = wxp.tile([128, 1024], bf16, tag="w")
                        nc.scalar.activation(w_t[:], m_t[:], AF.Exp)
                        for h in range(2):
                            jt = jt2 * 2 + h
                            for ic in range(4):
                                nc.tensor.matmul(
                                    Us[ic][:],
                                    w_t[:, h * PANEL + ic * 128: h * PANEL + (ic + 1) * 128],
                                    xt_all[:, jt * XW:(jt + 1) * XW],
                                    start=(jt == 0),
                                    stop=(jt == NJT - 1),
                                )
                    for ic in range(4):
                        rec = finp.tile([128, 1], f32, tag="rec")
                        nc.vector.reciprocal(rec[:], Us[ic][:, HC:HC + 1])
                        o_t = finp.tile([128, HC], f32, tag="o")
                        nc.vector.tensor_scalar_mul(o_t[:], Us[ic][:, 0:HC], rec[:])
                        r0 = p * PANEL + ic * 128
                        nc.sync.dma_start(out=outd[r0:r0 + 128, :], in_=o_t[:])

    nc.compile()
    _CACHE["nc"] = nc
    return nc


def _in_maps(regional_means, adj, W, b, a):
    regional_means = np.ascontiguousarray(regional_means, dtype=np.float32)
    adj = np.ascontiguousarray(adj, dtype=np.float32)
    W = np.ascontiguousarray(W, dtype=np.float32)
    b = np.asarray(b, dtype=np.float32)
    a = np.asarray(a, dtype=np.float32)
    brow = np.concatenate([b, [1.0, 0.0]]).reshape(1, XW + 1).astype(np.float32)
    maps = []
    for c in range(NCORES):
        bb, hf = divmod(c, 2)
        i0 = hf * ROWS
        maps.append(
            {
                "rm": regional_means[bb],
                "rmo": np.ascontiguousarray(regional_means[bb, i0:i0 + ROWS]),
                "adjs": np.ascontiguousarray(adj[bb, i0:i0 + ROWS]),
                "Wt": W,
                "brow": brow,
                "bcol": b.reshape(HC, 1),
                "asrc": np.ascontiguousarray(a[:HC].reshape(HC, 1)),
                "adst": np.ascontiguousarray(a[HC:].reshape(HC, 1)),
            }
        )
    return maps


def kernel(regional_means, adj, W, b, a):
    from concourse.bass_utils import run_bass_kernel_spmd

    nc = _build()
    maps = _in_maps(regional_means, adj, W, b, a)
    res = run_bass_kernel_spmd(nc, maps, core_ids=list(range(NCORES)))
    out = np.empty((B, N, HC), np.float32)
    for c in range(NCORES):
        bb, hf = divmod(c, 2)
        out[bb, hf * ROWS:(hf + 1) * ROWS] = res.results[c]["out"]
    return out


# revision 7
# speedup vs baseline: 2.5642x; 2.5642x over previous
"""GAT message-passing kernel for 8 Trainium2 NeuronCores.

Problem (hardcoded shapes): B=4, N=4096, Cin=200, HC=128.
    x   = rm @ W.T + b                      (B, N, HC)
    e   = (x@a_src)[:, :, None] + (x@a_dst)[:, None, :]
    e   = leaky_relu(e * adj, 0.2)
    out = softmax(e, -1) @ x                (B, N, HC)

Sharding: 8 cores = batch (4) x query-row halves (2). Each core owns
adj[b, i0:i0+2048, :] (33.5 MB) and produces out[b, i0:i0+2048, :].

Per-core algorithm (transposed score layout [j partitions, i free]):
  - scores decompose: src_i = rm_i . (W.T a_src) + b.(a_src+a_dst),
    dst_j = rm_j . (W.T a_dst);  e_ij = src_i + dst_j.
  - leaky_relu is positively homogeneous and adj in {0,1}, so
    lrelu(e*adj) = adj * lrelu(e): one fused custom DVE op computes
    m = adjT * lrelu(src_bcast + dst) in a single 1x pass.
  - w = exp(m) on ScalarE (exp(0)=1 for masked entries matches the
    reference's mask-by-multiplication semantics).
  - aggregation: U[i, 0:128] = sum_j w_ij * x[j, :], D[i] = U[i, 128]
    via one bf16 matmul chain against x~ = [x | 1]; out = U / D.
  - adj is cast fp32->bf16 during the DMA load (values 0/1 exact) and
    transposed on the PE (1 cyc/row) into PSUM for the DVE mask op.
"""

import sys

if "/opt/trn_rl_repo" not in sys.path:
    sys.path.insert(0, "/opt/trn_rl_repo")

import numpy as np

B, N, Cin, HC = 4, 4096, 200, 128
ROWS = N // 2  # rows of adj per core
NEG_SLOPE = 0.2
NCORES = 8
C1 = 128              # first Cin chunk
C2 = Cin - C1         # second Cin chunk (72)
XW = HC + 1           # x~ width (129)
NJT = N // 128        # 32 j-tiles
PANEL = 512
NPANEL = ROWS // PANEL  # 4
NOWN = ROWS // 128      # 16 own i-tiles

_CACHE = {}


def _register_custom_op():
    """Fused DVE op: out = in1 * leaky_relu(in0 + s0, slope=s1).

    in0 = src broadcast tile (stream), s0 = dst per-partition scalar,
    in1 = transposed adj tile (stream, PSUM), s1 = slope immediate.
    """
    import concourse.dve_ops as dve_ops
    from concourse.dve_spec import Spec, Src0, Src1, C0, C1 as SC1, maxx, lower
    from concourse.dve_uop import DveOpSpec

    name = "GAT_MASKED_LRELU_ANT"
    for op in dve_ops.OPS:
        if op.name == name:
            return op
    _t = Src0 + C0
    body = maxx(_t, _t * SC1) * Src1
    spec = Spec(
        body=body,
        reference=lambda in0, in1, s0, s1, imm2: np.maximum(
            in0 + s0, (in0 + s0) * s1
        )
        * in1,
    )
    row = dve_ops._CUSTOM_DVE_ROW_BASE + len(dve_ops.OPS)
    shas = {}
    for ver in ("v3", "v4"):
        uops = lower(spec, ver=ver)
        shas[ver] = DveOpSpec(name=name, opcode=row, uops=uops, rd1_en=True).sha(ver)
    op = dve_ops.DveOp(name, spec, subdim=False, uops_sha=shas)
    dve_ops.OPS.append(op)
    dve_ops._SUB_OPCODE_FOR_NAME[name] = row
    dve_ops.CUSTOM_DVE_SPECS[name] = spec
    return op


def _build(reps=1):
    if ("nc", reps) in _CACHE:
        return _CACHE[("nc", reps)]

    import concourse.mybir as mybir
    import concourse.tile as tile
    from concourse import bacc
    from concourse.masks import make_identity

    GAT_OP = _register_custom_op()
    f32 = mybir.dt.float32
    bf16 = mybir.dt.bfloat16
    AF = mybir.ActivationFunctionType

    nc = bacc.Bacc("TRN2", target_bir_lowering=False, debug=False, num_devices=NCORES)

    rm = nc.dram_tensor("rm", [N, Cin], f32, kind="ExternalInput").ap()
    rmo = nc.dram_tensor("rmo", [ROWS, Cin], f32, kind="ExternalInput").ap()
    adjs = nc.dram_tensor("adjs", [ROWS, N], f32, kind="ExternalInput").ap()
    Wd = nc.dram_tensor("Wt", [HC, Cin], f32, kind="ExternalInput").ap()
    brow = nc.dram_tensor("brow", [1, XW + 1], f32, kind="ExternalInput").ap()
    bcol = nc.dram_tensor("bcol", [HC, 1], f32, kind="ExternalInput").ap()
    asrc = nc.dram_tensor("asrc", [HC, 1], f32, kind="ExternalInput").ap()
    adst = nc.dram_tensor("adst", [HC, 1], f32, kind="ExternalInput").ap()
    outd = nc.dram_tensor("out", [ROWS, HC], f32, kind="ExternalOutput").ap()

    with tile.TileContext(nc) as tc:
      for _rep in range(reps):
        with (
            tc.tile_pool(name="const", bufs=1) as cp,
            tc.tile_pool(name="persist", bufs=1) as pp,
        ):
            # ---------- constants ----------
            ident_f = cp.tile([128, 128], f32)
            make_identity(nc, ident_f[:])
            ident_b = cp.tile([128, 128], bf16)
            nc.vector.tensor_copy(ident_b[:], ident_f[:])
            ones_row = cp.tile([1, 128], f32)
            nc.gpsimd.memset(ones_row[:], 1.0)
            W_sb = cp.tile([128, Cin], f32)
            nc.sync.dma_start(out=W_sb[:], in_=Wd)
            brow_sb = cp.tile([1, XW + 1], f32)
            nc.sync.dma_start(out=brow_sb[:], in_=brow)
            bcol_sb = cp.tile([HC, 1], f32)
            nc.sync.dma_start(out=bcol_sb[:], in_=bcol)
            asrc_sb = cp.tile([HC, 1], f32)
            nc.sync.dma_start(out=asrc_sb[:], in_=asrc)
            adst_sb = cp.tile([HC, 1], f32)
            nc.sync.dma_start(out=adst_sb[:], in_=adst)

            with tc.tile_pool(name="setup_ps", bufs=2, space="PSUM") as sps:
                # W^T (two chunks packed side by side: [c0:128, d] | [c128:200, d])
                wt_ps = sps.tile([128, 256], f32, tag="wt")
                nc.tensor.transpose(wt_ps[:, 0:128], W_sb[:, 0:C1], ident_f[:])
                nc.tensor.transpose(wt_ps[0:C2, 128:256], W_sb[:, C1:Cin], ident_f[:])
                WT_sb = cp.tile([128, 256], f32)
                nc.scalar.copy(WT_sb[:, 0:128], wt_ps[:, 0:128])
                nc.scalar.copy(WT_sb[0:C2, 128:256], wt_ps[0:C2, 128:256])

                # w_src / w_dst = W^T a (cols: src_lo, src_hi, dst_lo, dst_hi)
                wv_ps = sps.tile([128, 4], f32, tag="wv")
                nc.tensor.matmul(wv_ps[:, 0:1], W_sb[:, 0:C1], asrc_sb[:], start=True, stop=True)
                nc.tensor.matmul(wv_ps[0:C2, 1:2], W_sb[:, C1:Cin], asrc_sb[:], start=True, stop=True)
                nc.tensor.matmul(wv_ps[:, 2:3], W_sb[:, 0:C1], adst_sb[:], start=True, stop=True)
                nc.tensor.matmul(wv_ps[0:C2, 3:4], W_sb[:, C1:Cin], adst_sb[:], start=True, stop=True)
                wv_sb = cp.tile([128, 4], f32)
                nc.scalar.copy(wv_sb[:, 0:1], wv_ps[:, 0:1])
                nc.scalar.copy(wv_sb[0:C2, 1:2], wv_ps[0:C2, 1:2])
                nc.scalar.copy(wv_sb[:, 2:3], wv_ps[:, 2:3])
                nc.scalar.copy(wv_sb[0:C2, 3:4], wv_ps[0:C2, 3:4])

                # scalar const C = b.(a_src + a_dst), folded into src
                c_ps = sps.tile([1, 1], f32, tag="c")
                nc.tensor.matmul(c_ps[:], bcol_sb[:], asrc_sb[:], start=True, stop=False)
                nc.tensor.matmul(c_ps[:], bcol_sb[:], adst_sb[:], start=False, stop=True)
                c_sb = cp.tile([1, 1], f32)
                nc.scalar.copy(c_sb[:], c_ps[:])

            # ---------- x~, dst, src ----------
            xt_all = pp.tile([128, NJT * XW], bf16)   # x~ = [x | 1] per j-tile
            dst_all = pp.tile([128, NJT], f32)        # dst column per j-tile
            src_row = pp.tile([1, ROWS], f32)
            src_bc = pp.tile([128, ROWS], f32)        # src broadcast along partitions

            with (
                tc.tile_pool(name="xin", bufs=3) as xp,
                tc.tile_pool(name="xT", bufs=2) as xtp,
                tc.tile_pool(name="x_ps", bufs=2, space="PSUM") as xps,
                tc.tile_pool(name="x_ps2", bufs=2, space="PSUM") as xps2,
            ):
                for n in range(NJT):
                    rm_t = xp.tile([128, Cin], f32, tag="rm")
                    nc.sync.dma_start(out=rm_t[:], in_=rm[n * 128:(n + 1) * 128, :])
                    rT_ps = xps.tile([128, 256], f32, tag="rT")
                    nc.tensor.transpose(rT_ps[:, 0:128], rm_t[:, 0:C1], ident_f[:])
                    nc.tensor.transpose(rT_ps[0:C2, 128:256], rm_t[:, C1:Cin], ident_f[:])
                    rT_sb = xtp.tile([128, 256], f32, tag="rTs")
                    nc.scalar.copy(rT_sb[:, 0:128], rT_ps[:, 0:128])
                    nc.scalar.copy(rT_sb[0:C2, 128:256], rT_ps[0:C2, 128:256])

                    # cols 0:129 = x~ = [x | 1]; col 129 = dst.  One
                    # accumulation group: the K=1 bias matmul seeds every
                    # column (b | 1 | 0), everything else accumulates.
                    x_ps = xps2.tile([128, XW + 1], f32, tag="xps")
                    nc.tensor.matmul(x_ps[:], ones_row[:], brow_sb[:], start=True, stop=False)
                    nc.tensor.matmul(x_ps[:, 0:HC], rT_sb[:, 0:128], WT_sb[:, 0:128], start=False, stop=False)
                    nc.tensor.matmul(x_ps[:, 0:HC], rT_sb[0:C2, 128:256], WT_sb[0:C2, 128:256], start=False, stop=False)
                    nc.tensor.matmul(x_ps[:, XW:XW + 1], rT_sb[:, 0:128], wv_sb[:, 2:3], start=False, stop=False)
                    nc.tensor.matmul(x_ps[:, XW:XW + 1], rT_sb[0:C2, 128:256], wv_sb[0:C2, 3:4], start=False, stop=True)
                    nc.vector.tensor_copy(xt_all[:, n * XW:(n + 1) * XW], x_ps[:, 0:XW])
                    nc.vector.tensor_copy(dst_all[:, n:n + 1], x_ps[:, XW:XW + 1])

                for k in range(NOWN):
                    ro_t = xp.tile([128, Cin], f32, tag="rm")
                    nc.sync.dma_start(out=ro_t[:], in_=rmo[k * 128:(k + 1) * 128, :])
                    roT_ps = xps.tile([128, 256], f32, tag="rT")
                    nc.tensor.transpose(roT_ps[:, 0:128], ro_t[:, 0:C1], ident_f[:])
                    nc.tensor.transpose(roT_ps[0:C2, 128:256], ro_t[:, C1:Cin], ident_f[:])
                    roT_sb = xtp.tile([128, 256], f32, tag="rTs")
                    nc.scalar.copy(roT_sb[:, 0:128], roT_ps[:, 0:128])
                    nc.scalar.copy(roT_sb[0:C2, 128:256], roT_ps[0:C2, 128:256])

                    s_ps = xps2.tile([1, 128], f32, tag="xps", name="s_ps")
                    nc.tensor.matmul(s_ps[:], wv_sb[:, 0:1], roT_sb[:, 0:128], start=True, stop=False)
                    nc.tensor.matmul(s_ps[:], wv_sb[0:C2, 1:2], roT_sb[0:C2, 128:256], start=False, stop=False)
                    nc.tensor.matmul(s_ps[:], c_sb[:], ones_row[:], start=False, stop=True)
                    nc.scalar.copy(src_row[:, k * 128:(k + 1) * 128], s_ps[:])

                for q in range(ROWS // 512):
                    sb_ps = xps.tile([128, 512], f32, tag="rT", name="sb_ps")
                    nc.tensor.matmul(sb_ps[:], ones_row[:], src_row[:, q * 512:(q + 1) * 512], start=True, stop=True)
                    nc.vector.tensor_copy(src_bc[:, q * 512:(q + 1) * 512], sb_ps[:])

            # ---------- main loop ----------
            with (
                tc.tile_pool(name="adj", bufs=8) as adjp,
                tc.tile_pool(name="mbuf", bufs=2) as mwp,
                tc.tile_pool(name="wbuf", bufs=2) as wxp,
                tc.tile_pool(name="fin", bufs=4) as finp,
                tc.tile_pool(name="U_ps", bufs=4, space="PSUM") as upsp,
                tc.tile_pool(name="aT_ps", bufs=2, space="PSUM") as atp,
            ):
                for p in range(NPANEL):
                    strips = []
                    for s in range(4):
                        at = adjp.tile([128, N], bf16, tag="adj")
                        r0 = p * PANEL + s * 128
                        nc.gpsimd.dma_start(out=at[:], in_=adjs[r0:r0 + 128, :])
                        strips.append(at)
                    Us = [
                        upsp.tile([128, XW], f32, tag="U", name=f"U_{p}_{i}")
                        for i in range(4)
                    ]
                    for jt2 in range(NJT // 2):
                        m_t = mwp.tile([128, 1024], f32, tag="m")
                        for h in range(2):
                            jt = jt2 * 2 + h
                            aT = atp.tile([128, PANEL], bf16, tag="aT")
                            for s in range(4):
                                nc.tensor.transpose(
                                    aT[:, s * 128:(s + 1) * 128],
                                    strips[s][:, jt * 128:(jt + 1) * 128],
                                    ident_b[:],
                                )
                            nc.vector._custom_dve(
                                GAT_OP,
                                out=m_t[:, h * PANEL:(h + 1) * PANEL],
                                in0=src_bc[:, p * PANEL:(p + 1) * PANEL],
                                in1=aT[:],
                                s0=dst_all[:, jt:jt + 1],
                                s1=NEG_SLOPE,
                            )
                        w_t = wxp.tile([128, 1024], bf16, tag="w")
                        nc.scalar.activation(w_t[:], m_t[:], AF.Exp)
                        for h in range(2):
                            jt = jt2 * 2 + h
                            for ic in range(4):
                                nc.tensor.matmul(
                                    Us[ic][:],
                                    w_t[:, h * PANEL + ic * 128: h * PANEL + (ic + 1) * 128],
                                    xt_all[:, jt * XW:(jt + 1) * XW],
                                    start=(jt == 0),
                                    stop=(jt == NJT - 1),
                                )
                    for ic in range(4):
                        rec = finp.tile([128, 1], f32, tag="rec")
                        nc.vector.reciprocal(rec[:], Us[ic][:, HC:HC + 1])
                        o_t = finp.tile([128, HC], f32, tag="o")
                        nc.vector.tensor_scalar_mul(o_t[:], Us[ic][:, 0:HC], rec[:])
                        r0 = p * PANEL + ic * 128
                        nc.sync.dma_start(out=outd[r0:r0 + 128, :], in_=o_t[:])

    nc.compile()
    _CACHE[("nc", reps)] = nc
    return nc


def _in_maps(regional_means, adj, W, b, a):
    regional_means = np.ascontiguousarray(regional_means, dtype=np.float32)
    adj = np.ascontiguousarray(adj, dtype=np.float32)
    W = np.ascontiguousarray(W, dtype=np.float32)
    b = np.asarray(b, dtype=np.float32)
    a = np.asarray(a, dtype=np.float32)
    brow = np.concatenate([b, [1.0, 0.0]]).reshape(1, XW + 1).astype(np.float32)
    maps = []
    for c in range(NCORES):
        bb, hf = divmod(c, 2)
        i0 = hf * ROWS
        maps.append(
            {
                "rm": regional_means[bb],
                "rmo": np.ascontiguousarray(regional_means[bb, i0:i0 + ROWS]),
                "adjs": np.ascontiguousarray(adj[bb, i0:i0 + ROWS]),
                "Wt": W,
                "brow": brow,
                "bcol": b.reshape(HC, 1),
                "asrc": np.ascontiguousarray(a[:HC].reshape(HC, 1)),
                "adst": np.ascontiguousarray(a[HC:].reshape(HC, 1)),
            }
        )
    return maps


def kernel(regional_means, adj, W, b, a):
    from concourse.bass_utils import run_bass_kernel_spmd

    nc = _build()
    maps = _in_maps(regional_means, adj, W, b, a)
    res = run_bass_kernel_spmd(nc, maps, core_ids=list(range(NCORES)))
    out = np.empty((B, N, HC), np.float32)
    for c in range(NCORES):
        bb, hf = divmod(c, 2)
        out[bb, hf * ROWS:(hf + 1) * ROWS] = res.results[c]["out"]
    return out
